# revision 11
# baseline (speedup 1.0000x reference)
"""Self-contained Trainium2 Bass kernel: causal multi-head attention.

Problem: B=2, S=2048, D=1024, H=16 (DK=64), f32, returns (output, attn).

Sharding over 8 NeuronCores: core c handles batch b = c//4 and the 4 heads
4*(c%4) .. 4*(c%4)+4 (data parallel on B, tensor parallel on heads).  Each
core computes its heads' QKV projections, causal attention (writing its slice
of the attention-probability tensor), and a partial output projection; the
host sums the 4 partial output projections per batch (TP unshard).

On-device layout is "transposed space": activations are [channel, seq], so
score tiles come out as s^T[k, q] and feed the P@V matmul with no on-chip
transposes; the host pre-transposes inputs and de-transposes outputs.

Compute dtype is fp16 (operands; all accumulation f32 in PSUM) — full PE rate
with hideable weight loads.  The causal mask is applied on the PE itself via
an identity-matmul accumulation (psum += I.T @ maskT, mask bias -60000 so exp
underflows to exactly 0).  Softmax denominators come free as a ones-column
appended to V; normalization is reciprocal + gpsimd partition_broadcast + one
multiply that also produces the f32 attention tile.  Only lower-triangle
[k,q] block-rows are computed; the rest relies on pre-zeroed output buffers.
A non-causal mask falls back to a general variant computing every block.
"""

import sys
import types

if "/opt/trn_rl_repo" not in sys.path:
    sys.path.insert(0, "/opt/trn_rl_repo")

import numpy as np


def _install_ntff_hook():
    """Recreate antenv.axon_hooks (missing in this image) so that
    run_bass_kernel_spmd(trace=True) can capture NTFF profiles."""
    if "antenv.axon_hooks" in sys.modules:
        return
    try:
        from trn_agent_boot.trn_boot import _ntff_profile_via_ctypes
    except ImportError:
        return
    try:
        hook = _ntff_profile_via_ctypes("/opt/axon/libaxon_pjrt.so")
    except OSError:
        hook = None
    mod = types.ModuleType("antenv.axon_hooks")
    mod.get_axon_ntff_profile_hook = lambda: hook
    mod.set_axon_ntff_profile_hook = lambda h: None
    sys.modules["antenv.axon_hooks"] = mod


_install_ntff_hook()

import concourse.bacc as bacc
import concourse.mybir as mybir
import concourse.tile as tile
from concourse import bass_utils

B, S, D, H = 2, 2048, 1024, 16
DK = D // H          # 64
NCORES = 8
HPC = 4              # heads per core
E = HPC * DK         # 256 proj channels per core
NEG = -60000.0       # fp16-representable; exp(0.125*(s+NEG)) == 0 in f32
NKT = S // 128       # 16 key tiles
NSB = S // 512       # 4 seq blocks
ND = D // 128        # 8 contraction slices

F32 = mybir.dt.float32
F16 = mybir.dt.float16
Exp = mybir.ActivationFunctionType.Exp

_cache = {}


def _build(causal: bool, dbg: bool = False):
    nc = bacc.Bacc("TRN2", target_bir_lowering=False, debug=False,
                   num_devices=NCORES)

    # ---- I/O (inputs fp16; outputs f32) ----
    xq = nc.dram_tensor("xq", [D, S], F16, kind="ExternalInput")   # q[b].T
    xk = nc.dram_tensor("xk", [D, S], F16, kind="ExternalInput")
    xv = nc.dram_tensor("xv", [D, S], F16, kind="ExternalInput")
    wq = nc.dram_tensor("wq", [D, E], F16, kind="ExternalInput")   # Wq[hsl,:].T
    wk = nc.dram_tensor("wk", [D, E], F16, kind="ExternalInput")
    wv = nc.dram_tensor("wv", [D, HPC * (DK + 1)], F16, kind="ExternalInput")
    wo = nc.dram_tensor("wo", [E, D], F16, kind="ExternalInput")   # Wo[:,hsl].T
    bq = nc.dram_tensor("bq", [1, E], F16, kind="ExternalInput")
    bk = nc.dram_tensor("bk", [1, E], F16, kind="ExternalInput")
    bv = nc.dram_tensor("bv", [1, HPC * (DK + 1)], F16, kind="ExternalInput")
    bo = nc.dram_tensor("bo", [1, D], F16, kind="ExternalInput")
    onesr = nc.dram_tensor("onesr", [1, 512], F16, kind="ExternalInput")
    ident = nc.dram_tensor("ident", [128, 128], F16, kind="ExternalInput")
    # mask windows, transposed ([k, q]); causal: per-kt 512-wide window
    if causal:
        maskw = nc.dram_tensor("maskw", [NKT, 128, 512], F16, kind="ExternalInput")
    else:
        maskw = nc.dram_tensor("maskw", [NKT, 128, S], F16, kind="ExternalInput")

    attnT = nc.dram_tensor("attnT", [S, HPC, S], F32, kind="ExternalOutput")
    outT = nc.dram_tensor("outT", [D, S], F32, kind="ExternalOutput")
    if dbg:
        dqh = nc.dram_tensor("dqh", [2, 128, S], F16, kind="ExternalOutput")
        dkh = nc.dram_tensor("dkh", [2, 128, S], F16, kind="ExternalOutput")
        dvh = nc.dram_tensor("dvh", [NKT, 128, HPC * (DK + 1)], F16,
                             kind="ExternalOutput")
        dctx = nc.dram_tensor("dctx", [2, 128, S], F16, kind="ExternalOutput")

    with tile.TileContext(nc) as tc:
        with tc.tile_pool(name="const", bufs=1) as constp:
            wo_t = [constp.tile([128, D], F16, tag=f"wo{i}", name=f"wo{i}")
                    for i in range(2)]
            for i in range(2):
                nc.gpsimd.dma_start(wo_t[i][:], wo[i * 128:(i + 1) * 128, :])
            bo_t = constp.tile([1, D], F16, tag="bo", name="bo")
            nc.gpsimd.dma_start(bo_t[:], bo[:])
            ones_t = constp.tile([1, 512], F16, tag="ones", name="ones")
            nc.gpsimd.dma_start(ones_t[:], onesr[:])
            id_t = constp.tile([128, 128], F16, tag="ident", name="ident")
            nc.gpsimd.dma_start(id_t[:], ident[:])

            qhT = [constp.tile([128, S], F16, tag=f"qhT{i}", name=f"qhT{i}")
                   for i in range(2)]
            khT = [constp.tile([128, S], F16, tag=f"khT{i}", name=f"khT{i}")
                   for i in range(2)]
            vha = [constp.tile([128, HPC * (DK + 1)], F16, tag=f"vha{k}",
                               name=f"vha{k}") for k in range(NKT)]
            ctx = [constp.tile([128, S], F16, tag=f"ctx{i}", name=f"ctx{i}")
                   for i in range(2)]

            # ---------------- phase 1: projections ----------------
            with (
                tc.tile_pool(name="wproj", bufs=1) as wprojp,
                tc.tile_pool(name="xs", bufs=6) as xsp,
                tc.tile_pool(name="pp", bufs=4, space="PSUM") as pp,
            ):
                wq_t = wprojp.tile([128, ND * E], F16, tag="wq", name="wq")
                wk_t = wprojp.tile([128, ND * E], F16, tag="wk", name="wk")
                wv_t = wprojp.tile([128, ND * HPC * (DK + 1)], F16, tag="wv", name="wv")
                for w_t, w, we in ((wq_t, wq, E), (wk_t, wk, E),
                                   (wv_t, wv, HPC * (DK + 1))):
                    nc.gpsimd.dma_start(
                        w_t[:].rearrange("p (t e) -> p t e", e=we),
                        w[:].rearrange("(t p) e -> p t e", t=ND))
                bq_t = wprojp.tile([1, E], F16, tag="bq", name="bq")
                bk_t = wprojp.tile([1, E], F16, tag="bk", name="bk")
                bv_t = wprojp.tile([1, HPC * (DK + 1)], F16, tag="bv", name="bv")
                nc.gpsimd.dma_start(bq_t[:], bq[:])
                nc.gpsimd.dma_start(bk_t[:], bk[:])
                nc.gpsimd.dma_start(bv_t[:], bv[:])

                # vh: out[s_tile(128), e]; lhsT = x^T[d, s_tile], rhs = wv[d, e]
                for sb in range(NSB):
                    xt = []
                    for d4 in range(2):
                        t = xsp.tile([128, 2048], F16, tag="xs", name="xs")
                        nc.sync.dma_start(
                            t[:].rearrange("p (t q) -> p t q", q=512),
                            xv[d4 * 512:(d4 + 1) * 512,
                               sb * 512:(sb + 1) * 512]
                            .rearrange("(t p) q -> p t q", t=4))
                        xt.append(t)
                    EA = HPC * (DK + 1)
                    for st in range(4):
                        kt = sb * 4 + st
                        ps = pp.tile([128, EA], F32, tag="pp", name="pp")
                        nc.tensor.matmul(ps[:], ones_t[0:1, 0:128], bv_t[:],
                                         start=True, stop=False)
                        for ds in range(ND):
                            nc.tensor.matmul(
                                ps[:],
                                xt[ds // 4][:, (ds % 4) * 512 + st * 128:
                                            (ds % 4) * 512 + (st + 1) * 128],
                                wv_t[:, ds * EA:(ds + 1) * EA],
                                start=False, stop=(ds == ND - 1))
                        nc.scalar.copy(vha[kt][:], ps[:])

                # kh^T then qh^T: out[e_tile(128), s]; lhsT = w[d, e] slice
                for src, w_t, b_t, dst in ((xk, wk_t, bk_t, khT),
                                           (xq, wq_t, bq_t, qhT)):
                    for sb in range(NSB):
                        xt = []
                        for d4 in range(2):  # 4 d-slices per tile
                            t = xsp.tile([128, 2048], F16, tag="xs", name="xs")
                            nc.sync.dma_start(
                                t[:].rearrange("p (t q) -> p t q", q=512),
                                src[d4 * 512:(d4 + 1) * 512,
                                    sb * 512:(sb + 1) * 512]
                                .rearrange("(t p) q -> p t q", t=4))
                            xt.append(t)
                        for hp in range(2):
                            ps = pp.tile([128, 512], F32, tag="pp", name="pp")
                            nc.tensor.matmul(
                                ps[:], b_t[0:1, hp * 128:(hp + 1) * 128],
                                ones_t[:], start=True, stop=False)
                            for ds in range(ND):
                                nc.tensor.matmul(
                                    ps[:],
                                    w_t[:, ds * E + hp * 128:
                                        ds * E + (hp + 1) * 128],
                                    xt[ds // 4][:, (ds % 4) * 512:
                                                (ds % 4 + 1) * 512],
                                    start=False, stop=(ds == ND - 1))
                            nc.scalar.copy(dst[hp][:, sb * 512:(sb + 1) * 512],
                                           ps[:])
            # ------------- phase 2+3: attention + output proj -------------
            with (
                tc.tile_pool(name="pt", bufs=18) as ptp,
                tc.tile_pool(name="stage", bufs=5) as stagep,
                tc.tile_pool(name="rbp", bufs=4) as rbp,
                tc.tile_pool(name="rrow", bufs=4) as rrowp,
                tc.tile_pool(name="mw", bufs=6) as mwp,
                tc.tile_pool(name="dscr", bufs=4, space="DRAM") as dscrp,
                tc.tile_pool(name="ostage", bufs=3) as ostagep,
                tc.tile_pool(name="pspair", bufs=2, space="PSUM") as pspair,
                tc.tile_pool(name="psctx", bufs=2, space="PSUM") as psctx,
                tc.tile_pool(name="po", bufs=2, space="PSUM") as pop,
            ):
                for qb in range(NSB):
                    nkt = 4 * qb + 4 if causal else NKT
                    masked = set(range(4 * qb, 4 * qb + 4)) if causal \
                        else set(range(NKT))
                    mt = {}
                    for kt in sorted(masked):
                        m = mwp.tile([128, 512], F16, tag="mw", name="mw")
                        if causal:
                            nc.gpsimd.dma_start(m[:], maskw[kt, :, :])
                        else:
                            nc.gpsimd.dma_start(
                                m[:], maskw[kt, :, qb * 512:(qb + 1) * 512])
                        mt[kt] = m

                    for hp in range(2):
                        psc = [psctx.tile([65, 512], F32, tag="psctx",
                                          name="psctx") for _ in range(2)]
                        kt_pts = []
                        for kt in range(nkt):
                            pss = pspair.tile([128, 1024], F32,
                                              tag="pspair", name="pspair")
                            # both heads' scores, adjacent + disjoint row groups
                            for h01 in range(2):
                                nc.tensor.matmul(
                                    pss[:, h01 * 512:(h01 + 1) * 512],
                                    khT[hp][64 * h01:64 * h01 + 64,
                                            kt * 128:(kt + 1) * 128],
                                    qhT[hp][64 * h01:64 * h01 + 64,
                                            qb * 512:(qb + 1) * 512],
                                    start=True, stop=(kt not in masked),
                                    tile_position=(64 * h01, 0))
                            if kt in masked:
                                for h01 in range(2):
                                    nc.tensor.matmul(
                                        pss[:, h01 * 512:(h01 + 1) * 512],
                                        id_t[:], mt[kt][:],
                                        start=False, stop=True)
                            ptt = ptp.tile([128, 1024], F16, tag="pt",
                                           name="pt")
                            nc.scalar.activation(ptt[:], pss[:], Exp,
                                                 scale=0.125)
                            for h01 in range(2):
                                lh = 2 * hp + h01
                                nc.tensor.matmul(
                                    psc[h01][0:65, :],
                                    vha[kt][:, 65 * lh:65 * lh + 65],
                                    ptt[:, h01 * 512:(h01 + 1) * 512],
                                    start=(kt == 0), stop=(kt == nkt - 1))
                            kt_pts.append((ptt, kt))

                        rbt = rbp.tile([128, 1024], F32, tag="rb", name="rb")
                        for h01 in range(2):
                            rrow = rrowp.tile([1, 512], F32, tag="rrow",
                                              name="rrow")
                            nc.vector.reciprocal(rrow[0:1, :], psc[h01][64:65, :])
                            dscr = dscrp.tile([1, 512], F32, tag="dscr",
                                              name="dscr")
                            nc.gpsimd.dma_start(dscr[:], rrow[0:1, :])
                            nc.scalar.dma_start(
                                rbt[:, h01 * 512:(h01 + 1) * 512],
                                dscr[:].to_broadcast((128, 512)))
                            nc.vector.tensor_mul(
                                ctx[hp][64 * h01:64 * h01 + 64,
                                        qb * 512:(qb + 1) * 512],
                                psc[h01][0:64, :],
                                rbt[0:64, h01 * 512:h01 * 512 + 512])
                        for i, (ptt, kt) in enumerate(kt_pts):
                            st = stagep.tile([128, 1024], F32, tag="stage",
                                             name="stage")
                            eng = nc.gpsimd if i % 3 == 2 else nc.vector
                            eng.tensor_mul(st[:], ptt[:], rbt[:])
                            nc.sync.dma_start(
                                attnT[kt * 128:(kt + 1) * 128,
                                      2 * hp:2 * hp + 2,
                                      qb * 512:(qb + 1) * 512],
                                st[:].rearrange("p (t q) -> p t q", q=512))

                    # output projection for this qb
                    for m in range(D // 128):
                        ps = pop.tile([128, 512], F32, tag="po", name="po")
                        nc.tensor.matmul(ps[:], bo_t[0:1, m * 128:(m + 1) * 128],
                                         ones_t[:], start=True, stop=False)
                        for i in range(2):
                            nc.tensor.matmul(
                                ps[:], wo_t[i][:, m * 128:(m + 1) * 128],
                                ctx[i][:, qb * 512:(qb + 1) * 512],
                                start=False, stop=(i == 1))
                        ost = ostagep.tile([128, 512], F32, tag="ostage",
                                           name="ostage")
                        nc.scalar.copy(ost[:], ps[:])
                        nc.sync.dma_start(outT[m * 128:(m + 1) * 128,
                                               qb * 512:(qb + 1) * 512], ost[:])

            if dbg:
                for i in range(2):
                    nc.sync.dma_start(dqh[i], qhT[i][:])
                    nc.sync.dma_start(dkh[i], khT[i][:])
                    nc.sync.dma_start(dctx[i], ctx[i][:])
                for kk in range(NKT):
                    nc.sync.dma_start(dvh[kk], vha[kk][:])

    nc.compile()
    return nc


def _get_nc(causal: bool):
    if causal not in _cache:
        _cache[causal] = _build(causal)
    return _cache[causal]


def _prep_inputs(q, k, v, attn_mask, Wq, bq, Wk, bk, Wv, bv, Wo, bo):
    m2 = np.asarray(attn_mask).reshape(S, S)
    causal = bool(np.array_equal((m2 != 0), np.tril(np.ones((S, S), bool))))

    if causal:
        maskw = np.zeros((NKT, 128, 512), np.float32)
        for kt in range(NKT):
            r = kt % 4
            maskw[kt, :, 0:128 * r] = NEG
            sub = m2[kt * 128:(kt + 1) * 128, kt * 128:(kt + 1) * 128]  # [q,k]
            maskw[kt, :, 128 * r:128 * (r + 1)] = \
                np.where(sub == 0, np.float32(NEG), np.float32(0.0)).T
    else:
        maskw = np.empty((NKT, 128, S), np.float32)
        for kt in range(NKT):
            sub = m2[:, kt * 128:(kt + 1) * 128]  # [q, k]
            maskw[kt] = np.where(sub == 0, np.float32(NEG), np.float32(0.0)).T
    maskw = maskw.astype(np.float16)

    xT = {}
    for name, x in (("xq", q), ("xk", k), ("xv", v)):
        xT[name] = [np.ascontiguousarray(np.asarray(x)[b].T).astype(np.float16)
                    for b in range(B)]

    onesr16 = np.ones((1, 512), np.float16)
    ident16 = np.eye(128, dtype=np.float16)

    Wq, Wk, Wv, Wo = (np.asarray(a, np.float32) for a in (Wq, Wk, Wv, Wo))
    bqv, bkv, bvv, bov = (np.asarray(a, np.float32) for a in (bq, bk, bv, bo))

    in_maps = []
    for c in range(NCORES):
        b = c // 4
        hs = slice(E * (c % 4), E * (c % 4 + 1))
        wvT = Wv[hs, :].T  # [D, 256]
        wv_aug = np.zeros((D, HPC * (DK + 1)), np.float16)
        bv_aug = np.zeros((1, HPC * (DK + 1)), np.float16)
        for h in range(HPC):
            wv_aug[:, (DK + 1) * h:(DK + 1) * h + DK] = \
                wvT[:, DK * h:DK * (h + 1)].astype(np.float16)
            bv_aug[0, (DK + 1) * h:(DK + 1) * h + DK] = \
                bvv[hs][DK * h:DK * (h + 1)].astype(np.float16)
            bv_aug[0, (DK + 1) * h + DK] = 1.0
        im = {
            "xq": xT["xq"][b], "xk": xT["xk"][b], "xv": xT["xv"][b],
            "wq": np.ascontiguousarray(Wq[hs, :].T).astype(np.float16),
            "wk": np.ascontiguousarray(Wk[hs, :].T).astype(np.float16),
            "wv": wv_aug,
            "wo": np.ascontiguousarray(Wo[:, hs].T).astype(np.float16),
            "bq": bqv[hs][None, :].astype(np.float16),
            "bk": bkv[hs][None, :].astype(np.float16),
            "bv": bv_aug,
            "bo": (bov[None, :].astype(np.float16) if c % 4 == 0
                   else np.zeros((1, D), np.float16)),
            "onesr": onesr16, "ident": ident16,
            "maskw": maskw,
        }
        in_maps.append(im)
    return causal, in_maps


def _gather(results):
    attn = np.empty((B, H, S, S), np.float32)
    output = np.empty((B, S, D), np.float32)
    for b in range(B):
        acc = None
        for g in range(4):
            c = b * 4 + g
            r = results[c]
            at = r["attnT"]
            for i in range(HPC):
                attn[b, HPC * (c % 4) + i] = at[:, i, :].T
            acc = r["outT"] if acc is None else acc + r["outT"]
        output[b] = acc.T
    return output, attn


def run(trace=False, **inputs):
    causal, in_maps = _prep_inputs(**inputs)
    nc = _get_nc(causal)
    res = bass_utils.run_bass_kernel_spmd(
        nc, in_maps, core_ids=list(range(NCORES)), trace=trace)
    output, attn = _gather(res.results)
    return (output, attn), res


def kernel(**inputs):
    (output, attn), _ = run(trace=False, **inputs)
    return output, attn


# revision 12
# speedup vs baseline: 1.0489x; 1.0489x over previous
"""Self-contained Trainium2 Bass kernel: causal multi-head attention.

Problem: B=2, S=2048, D=1024, H=16 (DK=64), f32, returns (output, attn).

Sharding over 8 NeuronCores: core c handles batch b = c//4 and the 4 heads
4*(c%4) .. 4*(c%4)+4 (data parallel on B, tensor parallel on heads).  Each
core computes its heads' QKV projections, causal attention (writing its slice
of the attention-probability tensor), and a partial output projection; the
host sums the 4 partial output projections per batch (TP unshard).

On-device layout is "transposed space": activations are [channel, seq], so
score tiles come out as s^T[k, q] and feed the P@V matmul with no on-chip
transposes; the host pre-transposes inputs and de-transposes outputs.

Compute dtype is fp16 (operands; all accumulation f32 in PSUM) — full PE rate
with hideable weight loads.  The causal mask is applied on the PE itself via
an identity-matmul accumulation (psum += I.T @ maskT, mask bias -60000 so exp
underflows to exactly 0).  Softmax denominators come free as a ones-column
appended to V; normalization is reciprocal + gpsimd partition_broadcast + one
multiply that also produces the f32 attention tile.  Only lower-triangle
[k,q] block-rows are computed; the rest relies on pre-zeroed output buffers.
A non-causal mask falls back to a general variant computing every block.
"""

import sys
import types

if "/opt/trn_rl_repo" not in sys.path:
    sys.path.insert(0, "/opt/trn_rl_repo")

import numpy as np


def _install_ntff_hook():
    """Recreate antenv.axon_hooks (missing in this image) so that
    run_bass_kernel_spmd(trace=True) can capture NTFF profiles."""
    if "antenv.axon_hooks" in sys.modules:
        return
    try:
        from trn_agent_boot.trn_boot import _ntff_profile_via_ctypes
    except ImportError:
        return
    try:
        hook = _ntff_profile_via_ctypes("/opt/axon/libaxon_pjrt.so")
    except OSError:
        hook = None
    mod = types.ModuleType("antenv.axon_hooks")
    mod.get_axon_ntff_profile_hook = lambda: hook
    mod.set_axon_ntff_profile_hook = lambda h: None
    sys.modules["antenv.axon_hooks"] = mod


_install_ntff_hook()

import concourse.bacc as bacc
import concourse.mybir as mybir
import concourse.tile as tile
from concourse import bass_utils

B, S, D, H = 2, 2048, 1024, 16
DK = D // H          # 64
NCORES = 8
HPC = 4              # heads per core
E = HPC * DK         # 256 proj channels per core
NEG = -60000.0       # fp16-representable; exp(0.125*(s+NEG)) == 0 in f32
NKT = S // 128       # 16 key tiles
NSB = S // 512       # 4 seq blocks
ND = D // 128        # 8 contraction slices

F32 = mybir.dt.float32
F16 = mybir.dt.float16
Exp = mybir.ActivationFunctionType.Exp

_cache = {}


def _build(causal: bool, dbg: bool = False):
    nc = bacc.Bacc("TRN2", target_bir_lowering=False, debug=False,
                   num_devices=NCORES)

    # ---- I/O (inputs fp16; outputs f32) ----
    xq = nc.dram_tensor("xq", [D, S], F16, kind="ExternalInput")   # q[b].T
    xk = nc.dram_tensor("xk", [D, S], F16, kind="ExternalInput")
    xv = nc.dram_tensor("xv", [D, S], F16, kind="ExternalInput")
    wq = nc.dram_tensor("wq", [D, E], F16, kind="ExternalInput")   # Wq[hsl,:].T
    wk = nc.dram_tensor("wk", [D, E], F16, kind="ExternalInput")
    wv = nc.dram_tensor("wv", [D, HPC * (DK + 1)], F16, kind="ExternalInput")
    wo = nc.dram_tensor("wo", [E, D], F16, kind="ExternalInput")   # Wo[:,hsl].T
    bq = nc.dram_tensor("bq", [128, 2], F32, kind="ExternalInput")
    bk = nc.dram_tensor("bk", [128, 2], F32, kind="ExternalInput")
    bv = nc.dram_tensor("bv", [1, HPC * (DK + 1)], F16, kind="ExternalInput")
    bo = nc.dram_tensor("bo", [128, 8], F32, kind="ExternalInput")
    onesr = nc.dram_tensor("onesr", [1, 512], F16, kind="ExternalInput")
    # mask windows, transposed ([k, q]); causal: per-kt 512-wide window
    if causal:
        maskw = nc.dram_tensor("maskw", [NKT, 128, 512], F16, kind="ExternalInput")
    else:
        maskw = nc.dram_tensor("maskw", [NKT, 128, S], F16, kind="ExternalInput")

    attnT = nc.dram_tensor("attnT", [S, HPC, S], F16, kind="ExternalOutput")
    outT = nc.dram_tensor("outT", [D, S], F32, kind="ExternalOutput")
    if dbg:
        dqh = nc.dram_tensor("dqh", [2, 128, S], F16, kind="ExternalOutput")
        dkh = nc.dram_tensor("dkh", [2, 128, S], F16, kind="ExternalOutput")
        dvh = nc.dram_tensor("dvh", [NKT, 128, HPC * (DK + 1)], F16,
                             kind="ExternalOutput")
        dctx = nc.dram_tensor("dctx", [2, 128, S], F16, kind="ExternalOutput")

    with tile.TileContext(nc) as tc:
        with tc.tile_pool(name="const", bufs=1) as constp:
            wo_t = [constp.tile([128, D], F16, tag=f"wo{i}", name=f"wo{i}")
                    for i in range(2)]
            for i in range(2):
                nc.gpsimd.dma_start(wo_t[i][:], wo[i * 128:(i + 1) * 128, :])
            bo_t = constp.tile([128, 8], F32, tag="bo", name="bo")
            nc.gpsimd.dma_start(bo_t[:], bo[:])
            ones_t = constp.tile([1, 512], F16, tag="ones", name="ones")
            nc.gpsimd.dma_start(ones_t[:], onesr[:])

            qhT = [constp.tile([128, S], F16, tag=f"qhT{i}", name=f"qhT{i}")
                   for i in range(2)]
            khT = [constp.tile([128, S], F16, tag=f"khT{i}", name=f"khT{i}")
                   for i in range(2)]
            vha = [constp.tile([128, HPC * (DK + 1)], F16, tag=f"vha{k}",
                               name=f"vha{k}") for k in range(NKT)]
            ctx = [constp.tile([128, S], F16, tag=f"ctx{i}", name=f"ctx{i}")
                   for i in range(2)]

            # ---------------- phase 1: projections ----------------
            with (
                tc.tile_pool(name="wproj", bufs=1) as wprojp,
                tc.tile_pool(name="xs", bufs=6) as xsp,
                tc.tile_pool(name="pp", bufs=4, space="PSUM") as pp,
            ):
                wq_t = wprojp.tile([128, ND * E], F16, tag="wq", name="wq")
                wk_t = wprojp.tile([128, ND * E], F16, tag="wk", name="wk")
                wv_t = wprojp.tile([128, ND * HPC * (DK + 1)], F16, tag="wv", name="wv")
                for w_t, w, we in ((wq_t, wq, E), (wk_t, wk, E),
                                   (wv_t, wv, HPC * (DK + 1))):
                    nc.gpsimd.dma_start(
                        w_t[:].rearrange("p (t e) -> p t e", e=we),
                        w[:].rearrange("(t p) e -> p t e", t=ND))
                bq_t = wprojp.tile([128, 2], F32, tag="bq", name="bq")
                bk_t = wprojp.tile([128, 2], F32, tag="bk", name="bk")
                bv_t = wprojp.tile([1, HPC * (DK + 1)], F16, tag="bv", name="bv")
                nc.gpsimd.dma_start(bq_t[:], bq[:])
                nc.gpsimd.dma_start(bk_t[:], bk[:])
                nc.gpsimd.dma_start(bv_t[:], bv[:])

                # vh: out[s_tile(128), e]; lhsT = x^T[d, s_tile], rhs = wv[d, e]
                for sb in range(NSB):
                    xt = []
                    for d4 in range(2):
                        t = xsp.tile([128, 2048], F16, tag="xs", name="xs")
                        nc.sync.dma_start(
                            t[:].rearrange("p (t q) -> p t q", q=512),
                            xv[d4 * 512:(d4 + 1) * 512,
                               sb * 512:(sb + 1) * 512]
                            .rearrange("(t p) q -> p t q", t=4))
                        xt.append(t)
                    EA = HPC * (DK + 1)
                    for st in range(4):
                        kt = sb * 4 + st
                        ps = pp.tile([128, EA], F32, tag="pp", name="pp")
                        nc.tensor.matmul(ps[:], ones_t[0:1, 0:128], bv_t[:],
                                         start=True, stop=False)
                        for ds in range(ND):
                            nc.tensor.matmul(
                                ps[:],
                                xt[ds // 4][:, (ds % 4) * 512 + st * 128:
                                            (ds % 4) * 512 + (st + 1) * 128],
                                wv_t[:, ds * EA:(ds + 1) * EA],
                                start=False, stop=(ds == ND - 1))
                        nc.vector.tensor_copy(vha[kt][:], ps[:])

                # kh^T then qh^T: out[e_tile(128), s]; lhsT = w[d, e] slice
                for src, w_t, b_t, dst in ((xk, wk_t, bk_t, khT),
                                           (xq, wq_t, bq_t, qhT)):
                    for sb in range(NSB):
                        xt = []
                        for d4 in range(2):  # 4 d-slices per tile
                            t = xsp.tile([128, 2048], F16, tag="xs", name="xs")
                            nc.sync.dma_start(
                                t[:].rearrange("p (t q) -> p t q", q=512),
                                src[d4 * 512:(d4 + 1) * 512,
                                    sb * 512:(sb + 1) * 512]
                                .rearrange("(t p) q -> p t q", t=4))
                            xt.append(t)
                        for hp in range(2):
                            ps = pp.tile([128, 512], F32, tag="pp", name="pp")
                            for ds in range(ND):
                                nc.tensor.matmul(
                                    ps[:],
                                    w_t[:, ds * E + hp * 128:
                                        ds * E + (hp + 1) * 128],
                                    xt[ds // 4][:, (ds % 4) * 512:
                                                (ds % 4 + 1) * 512],
                                    start=(ds == 0), stop=(ds == ND - 1))
                            nc.vector.tensor_scalar_add(
                                dst[hp][:, sb * 512:(sb + 1) * 512], ps[:],
                                b_t[:, hp:hp + 1])
            # ------------- phase 2+3: attention + output proj -------------
            with (
                tc.tile_pool(name="pt", bufs=18) as ptp,
                tc.tile_pool(name="stage", bufs=5) as stagep,
                tc.tile_pool(name="rbp", bufs=4) as rbp,
                tc.tile_pool(name="rrow", bufs=4) as rrowp,
                tc.tile_pool(name="mw", bufs=6) as mwp,
                tc.tile_pool(name="dscr", bufs=4, space="DRAM") as dscrp,
                tc.tile_pool(name="ostage", bufs=3) as ostagep,
                tc.tile_pool(name="pspair", bufs=2, space="PSUM") as pspair,
                tc.tile_pool(name="psctx", bufs=2, space="PSUM") as psctx,
                tc.tile_pool(name="po", bufs=2, space="PSUM") as pop,
            ):
                for qb in range(NSB):
                    nkt = 4 * qb + 4 if causal else NKT
                    masked = set(range(4 * qb, 4 * qb + 4)) if causal \
                        else set(range(NKT))
                    mt = {}
                    for kt in sorted(masked):
                        m = mwp.tile([128, 512], F16, tag="mw", name="mw")
                        if causal:
                            nc.gpsimd.dma_start(m[:], maskw[kt, :, :])
                        else:
                            nc.gpsimd.dma_start(
                                m[:], maskw[kt, :, qb * 512:(qb + 1) * 512])
                        mt[kt] = m

                    for hp in range(2):
                        psc = [psctx.tile([65, 512], F32, tag="psctx",
                                          name="psctx") for _ in range(2)]
                        kt_pts = []
                        for kt in range(nkt):
                            pss = pspair.tile([128, 1024], F32,
                                              tag="pspair", name="pspair")
                            # both heads' scores, adjacent + disjoint row groups
                            for h01 in range(2):
                                nc.tensor.matmul(
                                    pss[:, h01 * 512:(h01 + 1) * 512],
                                    khT[hp][64 * h01:64 * h01 + 64,
                                            kt * 128:(kt + 1) * 128],
                                    qhT[hp][64 * h01:64 * h01 + 64,
                                            qb * 512:(qb + 1) * 512],
                                    start=True, stop=True,
                                    tile_position=(64 * h01, 0))
                            if kt in masked:
                                for h01 in range(2):
                                    nc.vector.tensor_add(
                                        pss[:, h01 * 512:(h01 + 1) * 512],
                                        pss[:, h01 * 512:(h01 + 1) * 512],
                                        mt[kt][:])
                            ptt = ptp.tile([128, 1024], F16, tag="pt",
                                           name="pt")
                            nc.scalar.activation(ptt[:], pss[:], Exp,
                                                 scale=0.125)
                            for h01 in range(2):
                                lh = 2 * hp + h01
                                nc.tensor.matmul(
                                    psc[h01][0:65, :],
                                    vha[kt][:, 65 * lh:65 * lh + 65],
                                    ptt[:, h01 * 512:(h01 + 1) * 512],
                                    start=(kt == 0), stop=(kt == nkt - 1))
                            kt_pts.append((ptt, kt))

                        rbt = rbp.tile([128, 1024], F16, tag="rb", name="rb")
                        for h01 in range(2):
                            rrow = rrowp.tile([1, 512], F32, tag="rrow",
                                              name="rrow")
                            nc.vector.tensor_copy(rrow[0:1, :], psc[h01][64:65, :])
                            dscrA = dscrp.tile([1, 512], F32, tag="dscrA",
                                               name="dscrA")
                            nc.gpsimd.dma_start(dscrA[:], rrow[0:1, :])
                            den128 = rrowp.tile([128, 4], F32, tag="den128",
                                                name="den128")
                            nc.scalar.dma_start(
                                den128[:],
                                dscrA[0:1, :].rearrange("o (p c) -> (o p) c",
                                                        p=128))
                            rec16 = rrowp.tile([128, 4], F16, tag="rec16",
                                               name="rec16")
                            rec32 = rrowp.tile([128, 4], F32, tag="rec32",
                                               name="rec32")
                            nc.vector.reciprocal(rec32[:], den128[:])
                            nc.vector.tensor_copy(rec16[:], rec32[:])
                            dscrB = dscrp.tile([1, 512], F16, tag="dscrB",
                                               name="dscrB")
                            nc.gpsimd.dma_start(
                                dscrB[0:1, :].rearrange("o (p c) -> (o p) c",
                                                        p=128),
                                rec16[:])
                            nc.scalar.dma_start(
                                rbt[:, h01 * 512:(h01 + 1) * 512],
                                dscrB[:].to_broadcast((128, 512)))
                            nc.vector.tensor_mul(
                                ctx[hp][64 * h01:64 * h01 + 64,
                                        qb * 512:(qb + 1) * 512],
                                psc[h01][0:64, :],
                                rbt[0:64, h01 * 512:h01 * 512 + 512])
                        for i, (ptt, kt) in enumerate(kt_pts):
                            st = stagep.tile([128, 1024], F16, tag="stage",
                                             name="stage")
                            eng = nc.gpsimd if i % 3 == 2 else nc.vector
                            eng.tensor_mul(st[:], ptt[:], rbt[:])
                            nc.sync.dma_start(
                                attnT[kt * 128:(kt + 1) * 128,
                                      2 * hp:2 * hp + 2,
                                      qb * 512:(qb + 1) * 512],
                                st[:].rearrange("p (t q) -> p t q", q=512))

                    # output projection for this qb
                    for m in range(D // 128):
                        ps = pop.tile([128, 512], F32, tag="po", name="po")
                        for i in range(2):
                            nc.tensor.matmul(
                                ps[:], wo_t[i][:, m * 128:(m + 1) * 128],
                                ctx[i][:, qb * 512:(qb + 1) * 512],
                                start=(i == 0), stop=(i == 1))
                        ost = ostagep.tile([128, 512], F32, tag="ostage",
                                           name="ostage")
                        nc.vector.tensor_scalar_add(ost[:], ps[:],
                                                    bo_t[:, m:m + 1])
                        nc.sync.dma_start(outT[m * 128:(m + 1) * 128,
                                               qb * 512:(qb + 1) * 512], ost[:])

            if dbg:
                for i in range(2):
                    nc.sync.dma_start(dqh[i], qhT[i][:])
                    nc.sync.dma_start(dkh[i], khT[i][:])
                    nc.sync.dma_start(dctx[i], ctx[i][:])
                for kk in range(NKT):
                    nc.sync.dma_start(dvh[kk], vha[kk][:])

    nc.compile()
    return nc


def _get_nc(causal: bool):
    if causal not in _cache:
        _cache[causal] = _build(causal)
    return _cache[causal]


def _prep_inputs(q, k, v, attn_mask, Wq, bq, Wk, bk, Wv, bv, Wo, bo):
    m2 = np.asarray(attn_mask).reshape(S, S)
    causal = bool(np.array_equal((m2 != 0), np.tril(np.ones((S, S), bool))))

    if causal:
        maskw = np.zeros((NKT, 128, 512), np.float32)
        for kt in range(NKT):
            r = kt % 4
            maskw[kt, :, 0:128 * r] = NEG
            sub = m2[kt * 128:(kt + 1) * 128, kt * 128:(kt + 1) * 128]  # [q,k]
            maskw[kt, :, 128 * r:128 * (r + 1)] = \
                np.where(sub == 0, np.float32(NEG), np.float32(0.0)).T
    else:
        maskw = np.empty((NKT, 128, S), np.float32)
        for kt in range(NKT):
            sub = m2[:, kt * 128:(kt + 1) * 128]  # [q, k]
            maskw[kt] = np.where(sub == 0, np.float32(NEG), np.float32(0.0)).T
    maskw = maskw.astype(np.float16)

    xT = {}
    for name, x in (("xq", q), ("xk", k), ("xv", v)):
        xT[name] = [np.ascontiguousarray(np.asarray(x)[b].T).astype(np.float16)
                    for b in range(B)]

    onesr16 = np.ones((1, 512), np.float16)

    Wq, Wk, Wv, Wo = (np.asarray(a, np.float32) for a in (Wq, Wk, Wv, Wo))
    bqv, bkv, bvv, bov = (np.asarray(a, np.float32) for a in (bq, bk, bv, bo))

    in_maps = []
    for c in range(NCORES):
        b = c // 4
        hs = slice(E * (c % 4), E * (c % 4 + 1))
        wvT = Wv[hs, :].T  # [D, 256]
        wv_aug = np.zeros((D, HPC * (DK + 1)), np.float16)
        bv_aug = np.zeros((1, HPC * (DK + 1)), np.float16)
        for h in range(HPC):
            wv_aug[:, (DK + 1) * h:(DK + 1) * h + DK] = \
                wvT[:, DK * h:DK * (h + 1)].astype(np.float16)
            bv_aug[0, (DK + 1) * h:(DK + 1) * h + DK] = \
                bvv[hs][DK * h:DK * (h + 1)].astype(np.float16)
            bv_aug[0, (DK + 1) * h + DK] = 1.0
        im = {
            "xq": xT["xq"][b], "xk": xT["xk"][b], "xv": xT["xv"][b],
            "wq": np.ascontiguousarray(Wq[hs, :].T).astype(np.float16),
            "wk": np.ascontiguousarray(Wk[hs, :].T).astype(np.float16),
            "wv": wv_aug,
            "wo": np.ascontiguousarray(Wo[:, hs].T).astype(np.float16),
            "bq": np.ascontiguousarray(bqv[hs].reshape(2, 128).T),
            "bk": np.ascontiguousarray(bkv[hs].reshape(2, 128).T),
            "bv": bv_aug,
            "bo": (np.ascontiguousarray(bov.reshape(8, 128).T)
                   if c % 4 == 0 else np.zeros((128, 8), np.float32)),
            "onesr": onesr16,
            "maskw": maskw,
        }
        in_maps.append(im)
    return causal, in_maps


def _gather(results):
    attn = np.empty((B, H, S, S), np.float32)
    output = np.empty((B, S, D), np.float32)
    for b in range(B):
        acc = None
        for g in range(4):
            c = b * 4 + g
            r = results[c]
            at = r["attnT"]
            for i in range(HPC):
                attn[b, HPC * (c % 4) + i] = at[:, i, :].T.astype(np.float32)
            acc = r["outT"] if acc is None else acc + r["outT"]
        output[b] = acc.T
    return output, attn


def run(trace=False, **inputs):
    causal, in_maps = _prep_inputs(**inputs)
    nc = _get_nc(causal)
    res = bass_utils.run_bass_kernel_spmd(
        nc, in_maps, core_ids=list(range(NCORES)), trace=trace)
    output, attn = _gather(res.results)
    return (output, attn), res


def kernel(**inputs):
    (output, attn), _ = run(trace=False, **inputs)
    return output, attn


# revision 13
# speedup vs baseline: 1.2075x; 1.1513x over previous
"""Self-contained Trainium2 Bass kernel: causal multi-head attention.

Problem: B=2, S=2048, D=1024, H=16 (DK=64), f32, returns (output, attn).

Sharding over 8 NeuronCores: core c handles batch b = c//4 and the 4 heads
4*(c%4) .. 4*(c%4)+4 (data parallel on B, tensor parallel on heads).  Each
core computes its heads' QKV projections, causal attention (writing its slice
of the attention-probability tensor), and a partial output projection; the
host sums the 4 partial output projections per batch (TP unshard).

On-device layout is "transposed space": activations are [channel, seq], so
score tiles come out as s^T[k, q] and feed the P@V matmul with no on-chip
transposes; the host pre-transposes inputs and de-transposes outputs.

Compute dtype is fp16 (operands; all accumulation f32 in PSUM) — full PE rate
with hideable weight loads.  The causal mask is applied on the PE itself via
an identity-matmul accumulation (psum += I.T @ maskT, mask bias -60000 so exp
underflows to exactly 0).  Softmax denominators come free as a ones-column
appended to V; normalization is reciprocal + gpsimd partition_broadcast + one
multiply that also produces the f32 attention tile.  Only lower-triangle
[k,q] block-rows are computed; the rest relies on pre-zeroed output buffers.
A non-causal mask falls back to a general variant computing every block.
"""

import sys
import types

if "/opt/trn_rl_repo" not in sys.path:
    sys.path.insert(0, "/opt/trn_rl_repo")

import numpy as np


def _install_ntff_hook():
    """Recreate antenv.axon_hooks (missing in this image) so that
    run_bass_kernel_spmd(trace=True) can capture NTFF profiles."""
    if "antenv.axon_hooks" in sys.modules:
        return
    try:
        from trn_agent_boot.trn_boot import _ntff_profile_via_ctypes
    except ImportError:
        return
    try:
        hook = _ntff_profile_via_ctypes("/opt/axon/libaxon_pjrt.so")
    except OSError:
        hook = None
    mod = types.ModuleType("antenv.axon_hooks")
    mod.get_axon_ntff_profile_hook = lambda: hook
    mod.set_axon_ntff_profile_hook = lambda h: None
    sys.modules["antenv.axon_hooks"] = mod


_install_ntff_hook()

import concourse.bacc as bacc
import concourse.mybir as mybir
import concourse.tile as tile
from concourse import bass_utils

B, S, D, H = 2, 2048, 1024, 16
DK = D // H          # 64
NCORES = 8
HPC = 4              # heads per core
E = HPC * DK         # 256 proj channels per core
NEG = -60000.0       # fp16-representable; exp(0.125*(s+NEG)) == 0 in f32
NKT = S // 128       # 16 key tiles
NSB = S // 512       # 4 seq blocks
ND = D // 128        # 8 contraction slices

F32 = mybir.dt.float32
F16 = mybir.dt.float16
Exp = mybir.ActivationFunctionType.Exp

_cache = {}


def _build(causal: bool, dbg: bool = False):
    nc = bacc.Bacc("TRN2", target_bir_lowering=False, debug=False,
                   num_devices=NCORES)

    # ---- I/O (inputs fp16; outputs f32) ----
    xq = nc.dram_tensor("xq", [D, S], F16, kind="ExternalInput")   # q[b].T
    xk = nc.dram_tensor("xk", [D, S], F16, kind="ExternalInput")
    xv = nc.dram_tensor("xv", [D, S], F16, kind="ExternalInput")
    wq = nc.dram_tensor("wq", [D, E], F16, kind="ExternalInput")   # Wq[hsl,:].T
    wk = nc.dram_tensor("wk", [D, E], F16, kind="ExternalInput")
    wv = nc.dram_tensor("wv", [D, HPC * (DK + 1)], F16, kind="ExternalInput")
    wo = nc.dram_tensor("wo", [E, D], F16, kind="ExternalInput")   # Wo[:,hsl].T
    bq = nc.dram_tensor("bq", [128, 2], F32, kind="ExternalInput")
    bk = nc.dram_tensor("bk", [128, 2], F32, kind="ExternalInput")
    bv = nc.dram_tensor("bv", [1, HPC * (DK + 1)], F16, kind="ExternalInput")
    bo = nc.dram_tensor("bo", [128, 8], F32, kind="ExternalInput")
    onesr = nc.dram_tensor("onesr", [1, 512], F16, kind="ExternalInput")
    ident = nc.dram_tensor("ident", [128, 128], F16, kind="ExternalInput")
    # mask windows, transposed ([k, q]); causal: per-kt 512-wide window
    if causal:
        maskw = nc.dram_tensor("maskw", [NKT, 128, 512], F16, kind="ExternalInput")
    else:
        maskw = nc.dram_tensor("maskw", [NKT, 128, S], F16, kind="ExternalInput")

    attnT = nc.dram_tensor("attnT", [S, HPC, S], F16, kind="ExternalOutput")
    outT = nc.dram_tensor("outT", [D, S], F32, kind="ExternalOutput")
    if dbg:
        dqh = nc.dram_tensor("dqh", [2, 128, S], F16, kind="ExternalOutput")
        dkh = nc.dram_tensor("dkh", [2, 128, S], F16, kind="ExternalOutput")
        dvh = nc.dram_tensor("dvh", [NKT, 128, HPC * (DK + 1)], F16,
                             kind="ExternalOutput")
        dctx = nc.dram_tensor("dctx", [2, 128, S], F16, kind="ExternalOutput")

    with tile.TileContext(nc) as tc:
        with tc.tile_pool(name="const", bufs=1) as constp:
            wo_t = [constp.tile([128, D], F16, tag=f"wo{i}", name=f"wo{i}")
                    for i in range(2)]
            for i in range(2):
                nc.gpsimd.dma_start(wo_t[i][:], wo[i * 128:(i + 1) * 128, :])
            bo_t = constp.tile([128, 8], F32, tag="bo", name="bo")
            nc.gpsimd.dma_start(bo_t[:], bo[:])
            ones_t = constp.tile([1, 512], F16, tag="ones", name="ones")
            nc.gpsimd.dma_start(ones_t[:], onesr[:])
            id_t = constp.tile([128, 128], F16, tag="ident", name="ident")
            nc.gpsimd.dma_start(id_t[:], ident[:])

            qhT = [constp.tile([128, S], F16, tag=f"qhT{i}", name=f"qhT{i}")
                   for i in range(2)]
            khT = [constp.tile([128, S], F16, tag=f"khT{i}", name=f"khT{i}")
                   for i in range(2)]
            vha = [constp.tile([128, HPC * (DK + 1)], F16, tag=f"vha{k}",
                               name=f"vha{k}") for k in range(NKT)]
            ctx = [constp.tile([128, S], F16, tag=f"ctx{i}", name=f"ctx{i}")
                   for i in range(2)]

            # ---------------- phase 1: projections ----------------
            with (
                tc.tile_pool(name="wproj", bufs=1) as wprojp,
                tc.tile_pool(name="xs", bufs=6) as xsp,
                tc.tile_pool(name="pp", bufs=4, space="PSUM") as pp,
            ):
                wq_t = wprojp.tile([128, ND * E], F16, tag="wq", name="wq")
                wk_t = wprojp.tile([128, ND * E], F16, tag="wk", name="wk")
                wv_t = wprojp.tile([128, ND * HPC * (DK + 1)], F16, tag="wv", name="wv")
                for w_t, w, we in ((wq_t, wq, E), (wk_t, wk, E),
                                   (wv_t, wv, HPC * (DK + 1))):
                    nc.gpsimd.dma_start(
                        w_t[:].rearrange("p (t e) -> p t e", e=we),
                        w[:].rearrange("(t p) e -> p t e", t=ND))
                bq_t = wprojp.tile([128, 2], F32, tag="bq", name="bq")
                bk_t = wprojp.tile([128, 2], F32, tag="bk", name="bk")
                bv_t = wprojp.tile([1, HPC * (DK + 1)], F16, tag="bv", name="bv")
                nc.gpsimd.dma_start(bq_t[:], bq[:])
                nc.gpsimd.dma_start(bk_t[:], bk[:])
                nc.gpsimd.dma_start(bv_t[:], bv[:])

                # vh: out[s_tile(128), e]; lhsT = x^T[d, s_tile], rhs = wv[d, e]
                for sb in range(NSB):
                    xt = []
                    for d4 in range(2):
                        t = xsp.tile([128, 2048], F16, tag="xs", name="xs")
                        nc.sync.dma_start(
                            t[:].rearrange("p (t q) -> p t q", q=512),
                            xv[d4 * 512:(d4 + 1) * 512,
                               sb * 512:(sb + 1) * 512]
                            .rearrange("(t p) q -> p t q", t=4))
                        xt.append(t)
                    EA = HPC * (DK + 1)
                    for st in range(4):
                        kt = sb * 4 + st
                        ps = pp.tile([128, EA], F32, tag="pp", name="pp")
                        nc.tensor.matmul(ps[:], ones_t[0:1, 0:128], bv_t[:],
                                         start=True, stop=False)
                        for ds in range(ND):
                            nc.tensor.matmul(
                                ps[:],
                                xt[ds // 4][:, (ds % 4) * 512 + st * 128:
                                            (ds % 4) * 512 + (st + 1) * 128],
                                wv_t[:, ds * EA:(ds + 1) * EA],
                                start=False, stop=(ds == ND - 1))
                        nc.vector.tensor_copy(vha[kt][:], ps[:])

                # kh^T then qh^T: out[e_tile(128), s]; lhsT = w[d, e] slice
                for src, w_t, b_t, dst in ((xk, wk_t, bk_t, khT),
                                           (xq, wq_t, bq_t, qhT)):
                    for sb in range(NSB):
                        xt = []
                        for d4 in range(2):  # 4 d-slices per tile
                            t = xsp.tile([128, 2048], F16, tag="xs", name="xs")
                            nc.sync.dma_start(
                                t[:].rearrange("p (t q) -> p t q", q=512),
                                src[d4 * 512:(d4 + 1) * 512,
                                    sb * 512:(sb + 1) * 512]
                                .rearrange("(t p) q -> p t q", t=4))
                            xt.append(t)
                        for hp in range(2):
                            ps = pp.tile([128, 512], F32, tag="pp", name="pp")
                            for ds in range(ND):
                                nc.tensor.matmul(
                                    ps[:],
                                    w_t[:, ds * E + hp * 128:
                                        ds * E + (hp + 1) * 128],
                                    xt[ds // 4][:, (ds % 4) * 512:
                                                (ds % 4 + 1) * 512],
                                    start=(ds == 0), stop=(ds == ND - 1))
                            nc.vector.tensor_scalar_add(
                                dst[hp][:, sb * 512:(sb + 1) * 512], ps[:],
                                b_t[:, hp:hp + 1])
            # ------------- phase 2+3: attention + output proj -------------
            with (
                tc.tile_pool(name="pt", bufs=32) as ptp,
                tc.tile_pool(name="stage", bufs=8) as stagep,
                tc.tile_pool(name="rbp", bufs=4) as rbp,
                tc.tile_pool(name="rrow", bufs=4) as rrowp,
                tc.tile_pool(name="mw", bufs=6) as mwp,
                tc.tile_pool(name="dscr", bufs=4, space="DRAM") as dscrp,
                tc.tile_pool(name="ostage", bufs=3) as ostagep,
                tc.tile_pool(name="cun", bufs=5) as cunp,
                tc.tile_pool(name="pspair", bufs=2, space="PSUM") as pspair,
                tc.tile_pool(name="psctx", bufs=3, space="PSUM") as psctx,
                tc.tile_pool(name="po", bufs=1, space="PSUM") as pop,
            ):
                for qb in range(NSB):
                    nkt = 4 * qb + 4 if causal else NKT
                    masked = set(range(4 * qb, 4 * qb + 4)) if causal \
                        else set(range(NKT))
                    mt = {}
                    for kt in sorted(masked):
                        m = mwp.tile([128, 512], F16, tag="mw", name="mw")
                        if causal:
                            nc.gpsimd.dma_start(m[:], maskw[kt, :, :])
                        else:
                            nc.gpsimd.dma_start(
                                m[:], maskw[kt, :, qb * 512:(qb + 1) * 512])
                        mt[kt] = m

                    for hp in range(2):
                        psc = [psctx.tile([65, 512], F32, tag="psctx",
                                          name="psctx") for _ in range(2)]
                        kt_pts = []
                        for kt in range(nkt):
                            # causal: left cols < diag are dead; narrow consumers
                            off = max(0, 128 * kt - 512 * qb) if causal else 0
                            pss = pspair.tile([128, 1024], F32,
                                              tag="pspair", name="pspair")
                            # both heads' scores, adjacent + disjoint row groups
                            for h01 in range(2):
                                nc.tensor.matmul(
                                    pss[:, h01 * 512:(h01 + 1) * 512],
                                    khT[hp][64 * h01:64 * h01 + 64,
                                            kt * 128:(kt + 1) * 128],
                                    qhT[hp][64 * h01:64 * h01 + 64,
                                            qb * 512:(qb + 1) * 512],
                                    start=True, stop=(kt not in masked),
                                    tile_position=(64 * h01, 0))
                            if kt in masked:
                                mwid = 128 if causal else 512
                                for h01 in range(2):
                                    nc.tensor.matmul(
                                        pss[:, h01 * 512 + off:
                                            h01 * 512 + off + mwid],
                                        id_t[:], mt[kt][:, off:off + mwid],
                                        start=False, stop=True)
                            ptt = ptp.tile([128, 1024], F16, tag="pt",
                                           name="pt")
                            nc.scalar.activation(ptt[:], pss[:], Exp,
                                                 scale=0.125)
                            for h01 in range(2):
                                lh = 2 * hp + h01
                                nc.tensor.matmul(
                                    psc[h01][0:65, off:512],
                                    vha[kt][:, 65 * lh:65 * lh + 65],
                                    ptt[:, h01 * 512 + off:(h01 + 1) * 512],
                                    start=(kt == 0), stop=(kt == nkt - 1))
                            kt_pts.append((ptt, kt, off))

                        rbt = rbp.tile([128, 1024], F16, tag="rb", name="rb")
                        for h01 in range(2):
                            rrow = rrowp.tile([1, 512], F32, tag="rrow",
                                              name="rrow")
                            nc.vector.tensor_copy(rrow[0:1, :], psc[h01][64:65, :])
                            cun = cunp.tile([64, 512], F16, tag="cun",
                                            name="cun")
                            nc.vector.tensor_copy(cun[:], psc[h01][0:64, :])
                            dscrA = dscrp.tile([1, 512], F32, tag="dscrA",
                                               name="dscrA")
                            nc.scalar.dma_start(dscrA[:], rrow[0:1, :])
                            den128 = rrowp.tile([128, 4], F32, tag="den128",
                                                name="den128")
                            nc.scalar.dma_start(
                                den128[:],
                                dscrA[0:1, :].rearrange("o (p c) -> (o p) c",
                                                        p=128))
                            rec16 = rrowp.tile([128, 4], F16, tag="rec16",
                                               name="rec16")
                            rec32 = rrowp.tile([128, 4], F32, tag="rec32",
                                               name="rec32")
                            nc.vector.reciprocal(rec32[:], den128[:])
                            nc.vector.tensor_copy(rec16[:], rec32[:])
                            dscrB = dscrp.tile([1, 512], F16, tag="dscrB",
                                               name="dscrB")
                            nc.scalar.dma_start(
                                dscrB[0:1, :].rearrange("o (p c) -> (o p) c",
                                                        p=128),
                                rec16[:])
                            nc.scalar.dma_start(
                                rbt[:, h01 * 512:(h01 + 1) * 512],
                                dscrB[:].to_broadcast((128, 512)))
                            nc.vector.tensor_mul(
                                ctx[hp][64 * h01:64 * h01 + 64,
                                        qb * 512:(qb + 1) * 512],
                                cun[:],
                                rbt[0:64, h01 * 512:h01 * 512 + 512])
                        for i, (ptt, kt, off) in enumerate(kt_pts):
                            st = stagep.tile([128, 1024], F16, tag="stage",
                                             name="stage")
                            eng = nc.gpsimd if i % 4 == 3 else nc.vector
                            eng.tensor_mul(st[:], ptt[:], rbt[:])
                            if off:
                                nc.sync.dma_start(
                                    attnT[kt * 128:(kt + 1) * 128,
                                          2 * hp:2 * hp + 2,
                                          qb * 512 + off:(qb + 1) * 512],
                                    st[:].rearrange("p (t q) -> p t q", q=512)
                                    [:, :, off:512])
                            else:
                                nc.sync.dma_start(
                                    attnT[kt * 128:(kt + 1) * 128,
                                          2 * hp:2 * hp + 2,
                                          qb * 512:(qb + 1) * 512],
                                    st[:].rearrange("p (t q) -> p t q", q=512))

                    # output projection for this qb
                    for m in range(D // 128):
                        ps = pop.tile([128, 512], F32, tag="po", name="po")
                        for i in range(2):
                            nc.tensor.matmul(
                                ps[:], wo_t[i][:, m * 128:(m + 1) * 128],
                                ctx[i][:, qb * 512:(qb + 1) * 512],
                                start=(i == 0), stop=(i == 1))
                        ost = ostagep.tile([128, 512], F32, tag="ostage",
                                           name="ostage")
                        nc.vector.tensor_scalar_add(ost[:], ps[:],
                                                    bo_t[:, m:m + 1])
                        nc.sync.dma_start(outT[m * 128:(m + 1) * 128,
                                               qb * 512:(qb + 1) * 512], ost[:])

            if dbg:
                for i in range(2):
                    nc.sync.dma_start(dqh[i], qhT[i][:])
                    nc.sync.dma_start(dkh[i], khT[i][:])
                    nc.sync.dma_start(dctx[i], ctx[i][:])
                for kk in range(NKT):
                    nc.sync.dma_start(dvh[kk], vha[kk][:])

    nc.compile()
    return nc


def _get_nc(causal: bool):
    if causal not in _cache:
        _cache[causal] = _build(causal)
    return _cache[causal]


def _prep_inputs(q, k, v, attn_mask, Wq, bq, Wk, bk, Wv, bv, Wo, bo):
    m2 = np.asarray(attn_mask).reshape(S, S)
    causal = bool(np.array_equal((m2 != 0), np.tril(np.ones((S, S), bool))))

    if causal:
        maskw = np.zeros((NKT, 128, 512), np.float32)
        for kt in range(NKT):
            r = kt % 4
            maskw[kt, :, 0:128 * r] = NEG
            sub = m2[kt * 128:(kt + 1) * 128, kt * 128:(kt + 1) * 128]  # [q,k]
            maskw[kt, :, 128 * r:128 * (r + 1)] = \
                np.where(sub == 0, np.float32(NEG), np.float32(0.0)).T
    else:
        maskw = np.empty((NKT, 128, S), np.float32)
        for kt in range(NKT):
            sub = m2[:, kt * 128:(kt + 1) * 128]  # [q, k]
            maskw[kt] = np.where(sub == 0, np.float32(NEG), np.float32(0.0)).T
    maskw = maskw.astype(np.float16)

    xT = {}
    for name, x in (("xq", q), ("xk", k), ("xv", v)):
        xT[name] = [np.ascontiguousarray(np.asarray(x)[b].T).astype(np.float16)
                    for b in range(B)]

    onesr16 = np.ones((1, 512), np.float16)
    ident16 = np.eye(128, dtype=np.float16)

    Wq, Wk, Wv, Wo = (np.asarray(a, np.float32) for a in (Wq, Wk, Wv, Wo))
    bqv, bkv, bvv, bov = (np.asarray(a, np.float32) for a in (bq, bk, bv, bo))

    in_maps = []
    for c in range(NCORES):
        b = c // 4
        hs = slice(E * (c % 4), E * (c % 4 + 1))
        wvT = Wv[hs, :].T  # [D, 256]
        wv_aug = np.zeros((D, HPC * (DK + 1)), np.float16)
        bv_aug = np.zeros((1, HPC * (DK + 1)), np.float16)
        for h in range(HPC):
            wv_aug[:, (DK + 1) * h:(DK + 1) * h + DK] = \
                wvT[:, DK * h:DK * (h + 1)].astype(np.float16)
            bv_aug[0, (DK + 1) * h:(DK + 1) * h + DK] = \
                bvv[hs][DK * h:DK * (h + 1)].astype(np.float16)
            bv_aug[0, (DK + 1) * h + DK] = 1.0
        im = {
            "xq": xT["xq"][b], "xk": xT["xk"][b], "xv": xT["xv"][b],
            "wq": np.ascontiguousarray(Wq[hs, :].T).astype(np.float16),
            "wk": np.ascontiguousarray(Wk[hs, :].T).astype(np.float16),
            "wv": wv_aug,
            "wo": np.ascontiguousarray(Wo[:, hs].T).astype(np.float16),
            "bq": np.ascontiguousarray(bqv[hs].reshape(2, 128).T),
            "bk": np.ascontiguousarray(bkv[hs].reshape(2, 128).T),
            "bv": bv_aug,
            "bo": (np.ascontiguousarray(bov.reshape(8, 128).T)
                   if c % 4 == 0 else np.zeros((128, 8), np.float32)),
            "onesr": onesr16, "ident": ident16,
            "maskw": maskw,
        }
        in_maps.append(im)
    return causal, in_maps


def _gather(results):
    attn = np.empty((B, H, S, S), np.float32)
    output = np.empty((B, S, D), np.float32)
    for b in range(B):
        acc = None
        for g in range(4):
            c = b * 4 + g
            r = results[c]
            at = r["attnT"]
            for i in range(HPC):
                attn[b, HPC * (c % 4) + i] = at[:, i, :].T.astype(np.float32)
            acc = r["outT"] if acc is None else acc + r["outT"]
        output[b] = acc.T
    return output, attn


def run(trace=False, **inputs):
    causal, in_maps = _prep_inputs(**inputs)
    nc = _get_nc(causal)
    res = bass_utils.run_bass_kernel_spmd(
        nc, in_maps, core_ids=list(range(NCORES)), trace=trace)
    output, attn = _gather(res.results)
    return (output, attn), res


def kernel(**inputs):
    (output, attn), _ = run(trace=False, **inputs)
    return output, attn


# revision 15
# speedup vs baseline: 1.3773x; 1.1406x over previous
"""Self-contained Trainium2 Bass kernel: causal multi-head attention.

Problem: B=2, S=2048, D=1024, H=16 (DK=64), f32, returns (output, attn).

Sharding over 8 NeuronCores: core c handles batch b = c//4 and the 4 heads
4*(c%4) .. 4*(c%4)+4 (data parallel on B, tensor parallel on heads).  Each
core computes its heads' QKV projections, causal attention (writing its slice
of the attention-probability tensor), and a partial output projection; the
host sums the 4 partial output projections per batch (TP unshard).

On-device layout is "transposed space": activations are [channel, seq], so
score tiles come out as s^T[k, q] and feed the P@V matmul with no on-chip
transposes; the host pre-transposes inputs and de-transposes outputs.

Compute dtype is fp16 (operands; all accumulation f32 in PSUM) — full PE rate
with hideable weight loads.  The causal mask is applied on the PE itself via
an identity-matmul accumulation (psum += I.T @ maskT, mask bias -60000 so exp
underflows to exactly 0).  Softmax denominators come free as a ones-column
appended to V; normalization is reciprocal + gpsimd partition_broadcast + one
multiply that also produces the f32 attention tile.  Only lower-triangle
[k,q] block-rows are computed; the rest relies on pre-zeroed output buffers.
A non-causal mask falls back to a general variant computing every block.
"""

import sys
import types

if "/opt/trn_rl_repo" not in sys.path:
    sys.path.insert(0, "/opt/trn_rl_repo")

import numpy as np


def _install_ntff_hook():
    """Recreate antenv.axon_hooks (missing in this image) so that
    run_bass_kernel_spmd(trace=True) can capture NTFF profiles."""
    if "antenv.axon_hooks" in sys.modules:
        return
    try:
        from trn_agent_boot.trn_boot import _ntff_profile_via_ctypes
    except ImportError:
        return
    try:
        hook = _ntff_profile_via_ctypes("/opt/axon/libaxon_pjrt.so")
    except OSError:
        hook = None
    mod = types.ModuleType("antenv.axon_hooks")
    mod.get_axon_ntff_profile_hook = lambda: hook
    mod.set_axon_ntff_profile_hook = lambda h: None
    sys.modules["antenv.axon_hooks"] = mod


_install_ntff_hook()

import concourse.bacc as bacc
import concourse.mybir as mybir
import concourse.tile as tile
from concourse import bass_utils

B, S, D, H = 2, 2048, 1024, 16
DK = D // H          # 64
NCORES = 8
HPC = 4              # heads per core
E = HPC * DK         # 256 proj channels per core
NEG = -60000.0       # fp16-representable; exp(0.125*(s+NEG)) == 0 in f32
NKT = S // 128       # 16 key tiles
NSB = S // 512       # 4 seq blocks
ND = D // 128        # 8 contraction slices

F32 = mybir.dt.float32
F16 = mybir.dt.float16
Exp = mybir.ActivationFunctionType.Exp
Log = mybir.ActivationFunctionType.Ln

_cache = {}


def _build(causal: bool, dbg: bool = False):
    nc = bacc.Bacc("TRN2", target_bir_lowering=False, debug=False,
                   num_devices=NCORES)

    # ---- I/O (inputs fp16; outputs f32) ----
    xq = nc.dram_tensor("xq", [D, S], F16, kind="ExternalInput")   # q[b].T
    xk = nc.dram_tensor("xk", [D, S], F16, kind="ExternalInput")
    xv = nc.dram_tensor("xv", [D, S], F16, kind="ExternalInput")
    wq = nc.dram_tensor("wq", [D, E], F16, kind="ExternalInput")   # Wq[hsl,:].T
    wk = nc.dram_tensor("wk", [D, E], F16, kind="ExternalInput")
    wv = nc.dram_tensor("wv", [D, HPC * (DK + 1)], F16, kind="ExternalInput")
    wo = nc.dram_tensor("wo", [E, D], F16, kind="ExternalInput")   # Wo[:,hsl].T
    bq = nc.dram_tensor("bq", [128, 2], F32, kind="ExternalInput")
    bk = nc.dram_tensor("bk", [128, 2], F32, kind="ExternalInput")
    bv = nc.dram_tensor("bv", [1, HPC * (DK + 1)], F16, kind="ExternalInput")
    bo = nc.dram_tensor("bo", [128, 8], F32, kind="ExternalInput")
    onesr = nc.dram_tensor("onesr", [1, 512], F16, kind="ExternalInput")
    ident = nc.dram_tensor("ident", [128, 128], F16, kind="ExternalInput")
    # mask windows, transposed ([k, q]); causal: per-kt 512-wide window
    if causal:
        maskw = nc.dram_tensor("maskw", [NKT, 128, 512], F16, kind="ExternalInput")
    else:
        maskw = nc.dram_tensor("maskw", [NKT, 128, S], F16, kind="ExternalInput")

    attnT = nc.dram_tensor("attnT", [S, HPC, S], F16, kind="ExternalOutput")
    outT = nc.dram_tensor("outT", [D, S], F32, kind="ExternalOutput")
    if dbg:
        dqh = nc.dram_tensor("dqh", [2, 128, S], F16, kind="ExternalOutput")
        dkh = nc.dram_tensor("dkh", [2, 128, S], F16, kind="ExternalOutput")
        dvh = nc.dram_tensor("dvh", [NKT, 128, HPC * (DK + 1)], F16,
                             kind="ExternalOutput")
        dctx = nc.dram_tensor("dctx", [2, 128, S], F16, kind="ExternalOutput")

    with tile.TileContext(nc) as tc:
        with tc.tile_pool(name="const", bufs=1) as constp:
            wo_t = [constp.tile([128, D], F16, tag=f"wo{i}", name=f"wo{i}")
                    for i in range(2)]
            for i in range(2):
                nc.gpsimd.dma_start(wo_t[i][:], wo[i * 128:(i + 1) * 128, :])
            bo_t = constp.tile([128, 8], F32, tag="bo", name="bo")
            nc.gpsimd.dma_start(bo_t[:], bo[:])
            ones_t = constp.tile([1, 512], F16, tag="ones", name="ones")
            nc.gpsimd.dma_start(ones_t[:], onesr[:])
            id_t = constp.tile([128, 128], F16, tag="ident", name="ident")
            nc.gpsimd.dma_start(id_t[:], ident[:])

            qhT = [constp.tile([128, S], F16, tag=f"qhT{i}", name=f"qhT{i}")
                   for i in range(2)]
            khT = [constp.tile([128, S], F16, tag=f"khT{i}", name=f"khT{i}")
                   for i in range(2)]
            vha = [constp.tile([128, HPC * (DK + 1)], F16, tag=f"vha{k}",
                               name=f"vha{k}") for k in range(NKT)]
            ctx = [constp.tile([128, S], F16, tag=f"ctx{i}", name=f"ctx{i}")
                   for i in range(2)]

            # ---------------- phase 1: projections ----------------
            with (
                tc.tile_pool(name="wproj", bufs=1) as wprojp,
                tc.tile_pool(name="xs", bufs=6) as xsp,
                tc.tile_pool(name="pp", bufs=4, space="PSUM") as pp,
            ):
                wq_t = wprojp.tile([128, ND * E], F16, tag="wq", name="wq")
                wk_t = wprojp.tile([128, ND * E], F16, tag="wk", name="wk")
                wv_t = wprojp.tile([128, ND * HPC * (DK + 1)], F16, tag="wv", name="wv")
                for w_t, w, we in ((wq_t, wq, E), (wk_t, wk, E),
                                   (wv_t, wv, HPC * (DK + 1))):
                    nc.gpsimd.dma_start(
                        w_t[:].rearrange("p (t e) -> p t e", e=we),
                        w[:].rearrange("(t p) e -> p t e", t=ND))
                bq_t = wprojp.tile([128, 2], F32, tag="bq", name="bq")
                bk_t = wprojp.tile([128, 2], F32, tag="bk", name="bk")
                bv_t = wprojp.tile([1, HPC * (DK + 1)], F16, tag="bv", name="bv")
                nc.gpsimd.dma_start(bq_t[:], bq[:])
                nc.gpsimd.dma_start(bk_t[:], bk[:])
                nc.gpsimd.dma_start(bv_t[:], bv[:])

                # vh: out[s_tile(128), e]; lhsT = x^T[d, s_tile], rhs = wv[d, e]
                for sb in range(NSB):
                    xt = []
                    for d4 in range(2):
                        t = xsp.tile([128, 2048], F16, tag="xs", name="xs")
                        nc.sync.dma_start(
                            t[:].rearrange("p (t q) -> p t q", q=512),
                            xv[d4 * 512:(d4 + 1) * 512,
                               sb * 512:(sb + 1) * 512]
                            .rearrange("(t p) q -> p t q", t=4))
                        xt.append(t)
                    EA = HPC * (DK + 1)
                    for st in range(4):
                        kt = sb * 4 + st
                        ps = pp.tile([128, EA], F32, tag="pp", name="pp")
                        nc.tensor.matmul(ps[:], ones_t[0:1, 0:128], bv_t[:],
                                         start=True, stop=False)
                        for ds in range(ND):
                            nc.tensor.matmul(
                                ps[:],
                                xt[ds // 4][:, (ds % 4) * 512 + st * 128:
                                            (ds % 4) * 512 + (st + 1) * 128],
                                wv_t[:, ds * EA:(ds + 1) * EA],
                                start=False, stop=(ds == ND - 1))
                        nc.vector.tensor_copy(vha[kt][:], ps[:])

                # kh^T then qh^T: out[e_tile(128), s]; lhsT = w[d, e] slice
                for src, w_t, b_t, dst in ((xk, wk_t, bk_t, khT),
                                           (xq, wq_t, bq_t, qhT)):
                    for sb in range(NSB):
                        xt = []
                        for d4 in range(2):  # 4 d-slices per tile
                            t = xsp.tile([128, 2048], F16, tag="xs", name="xs")
                            nc.sync.dma_start(
                                t[:].rearrange("p (t q) -> p t q", q=512),
                                src[d4 * 512:(d4 + 1) * 512,
                                    sb * 512:(sb + 1) * 512]
                                .rearrange("(t p) q -> p t q", t=4))
                            xt.append(t)
                        for hp in range(2):
                            ps = pp.tile([128, 512], F32, tag="pp", name="pp")
                            for ds in range(ND):
                                nc.tensor.matmul(
                                    ps[:],
                                    w_t[:, ds * E + hp * 128:
                                        ds * E + (hp + 1) * 128],
                                    xt[ds // 4][:, (ds % 4) * 512:
                                                (ds % 4 + 1) * 512],
                                    start=(ds == 0), stop=(ds == ND - 1))
                            nc.vector.tensor_scalar_add(
                                dst[hp][:, sb * 512:(sb + 1) * 512], ps[:],
                                b_t[:, hp:hp + 1])
            # ------------- phase 2+3: attention + output proj -------------
            with (
                tc.tile_pool(name="pt", bufs=32) as ptp,
                tc.tile_pool(name="stage", bufs=8) as stagep,
                tc.tile_pool(name="rbp", bufs=4) as rbp,
                tc.tile_pool(name="rrow", bufs=4) as rrowp,
                tc.tile_pool(name="mw", bufs=6) as mwp,
                tc.tile_pool(name="dscr", bufs=4, space="DRAM") as dscrp,
                tc.tile_pool(name="ostage", bufs=3) as ostagep,
                tc.tile_pool(name="cun", bufs=5) as cunp,
                tc.tile_pool(name="pspair", bufs=2, space="PSUM") as pspair,
                tc.tile_pool(name="psctx", bufs=3, space="PSUM") as psctx,
                tc.tile_pool(name="po", bufs=1, space="PSUM") as pop,
            ):
                for qb in range(NSB):
                    nkt = 4 * qb + 4 if causal else NKT
                    masked = set(range(4 * qb, 4 * qb + 4)) if causal \
                        else set(range(NKT))
                    mt = {}
                    for kt in sorted(masked):
                        m = mwp.tile([128, 512], F16, tag="mw", name="mw")
                        if causal:
                            nc.gpsimd.dma_start(m[:], maskw[kt, :, :])
                        else:
                            nc.gpsimd.dma_start(
                                m[:], maskw[kt, :, qb * 512:(qb + 1) * 512])
                        mt[kt] = m

                    norm_work = []
                    for hp in range(2):
                        psc = [psctx.tile([65, 512], F32, tag="psctx",
                                          name="psctx") for _ in range(2)]
                        kt_pts = []
                        for kt in range(nkt):
                            # causal: left cols < diag are dead; narrow consumers
                            off = max(0, 128 * kt - 512 * qb) if causal else 0
                            pss = pspair.tile([128, 1024], F32,
                                              tag="pspair", name="pspair")
                            # both heads' scores, adjacent + disjoint row groups
                            for h01 in range(2):
                                nc.tensor.matmul(
                                    pss[:, h01 * 512:(h01 + 1) * 512],
                                    khT[hp][64 * h01:64 * h01 + 64,
                                            kt * 128:(kt + 1) * 128],
                                    qhT[hp][64 * h01:64 * h01 + 64,
                                            qb * 512:(qb + 1) * 512],
                                    start=True, stop=(kt not in masked),
                                    tile_position=(64 * h01, 0))
                            if kt in masked:
                                mwid = 128 if causal else 512
                                for h01 in range(2):
                                    nc.tensor.matmul(
                                        pss[:, h01 * 512 + off:
                                            h01 * 512 + off + mwid],
                                        id_t[:], mt[kt][:, off:off + mwid],
                                        start=False, stop=True)
                            ptt = ptp.tile([128, 1024], F16, tag="pt",
                                           name="pt")
                            nc.scalar.activation(ptt[:], pss[:], Exp,
                                                 scale=0.125)
                            for h01 in range(2):
                                lh = 2 * hp + h01
                                nc.tensor.matmul(
                                    psc[h01][0:65, off:512],
                                    vha[kt][:, 65 * lh:65 * lh + 65],
                                    ptt[:, h01 * 512 + off:(h01 + 1) * 512],
                                    start=(kt == 0), stop=(kt == nkt - 1))
                            kt_pts.append((ptt, kt, off))

                        rbt = rbp.tile([128, 1024], F16, tag="rb", name="rb")
                        for h01 in range(2):
                            cun = cunp.tile([64, 512], F16, tag="cun",
                                            name="cun")
                            nc.vector.tensor_copy(cun[:], psc[h01][0:64, :])
                            lnrow = rrowp.tile([1, 512], F32, tag="lnrow",
                                               name="lnrow")
                            nc.scalar.activation(lnrow[0:1, :],
                                                 psc[h01][64:65, :], Log)
                            rrow16 = rrowp.tile([1, 512], F16, tag="rrow16",
                                                name="rrow16")
                            nc.scalar.activation(rrow16[0:1, :], lnrow[0:1, :],
                                                 Exp, scale=-1.0)
                            dscrB = dscrp.tile([1, 512], F16, tag="dscrB",
                                               name="dscrB")
                            nc.scalar.dma_start(dscrB[:], rrow16[0:1, :])
                            nc.scalar.dma_start(
                                rbt[:, h01 * 512:(h01 + 1) * 512],
                                dscrB[:].to_broadcast((128, 512)))
                            nc.vector.tensor_mul(
                                ctx[hp][64 * h01:64 * h01 + 64,
                                        qb * 512:(qb + 1) * 512],
                                cun[:],
                                rbt[0:64, h01 * 512:h01 * 512 + 512])
                        norm_work.append((hp, rbt, kt_pts))
                    # output projection for this qb (PE-only; overlaps norms)
                    for m in range(D // 128):
                        ps = pop.tile([128, 512], F32, tag="po", name="po")
                        for i in range(2):
                            nc.tensor.matmul(
                                ps[:], wo_t[i][:, m * 128:(m + 1) * 128],
                                ctx[i][:, qb * 512:(qb + 1) * 512],
                                start=(i == 0), stop=(i == 1))
                        ost = ostagep.tile([128, 512], F32, tag="ostage",
                                           name="ostage")
                        nc.vector.tensor_scalar_add(ost[:], ps[:],
                                                    bo_t[:, m:m + 1])
                        nc.sync.dma_start(outT[m * 128:(m + 1) * 128,
                                               qb * 512:(qb + 1) * 512], ost[:])
                    for hp, rbt, kt_pts in norm_work:
                        for i, (ptt, kt, off) in enumerate(kt_pts):
                            st = stagep.tile([128, 1024], F16, tag="stage",
                                             name="stage")
                            eng = nc.gpsimd if i % 4 == 3 else nc.vector
                            eng.tensor_mul(st[:], ptt[:], rbt[:])
                            if off:
                                nc.sync.dma_start(
                                    attnT[kt * 128:(kt + 1) * 128,
                                          2 * hp:2 * hp + 2,
                                          qb * 512 + off:(qb + 1) * 512],
                                    st[:].rearrange("p (t q) -> p t q", q=512)
                                    [:, :, off:512])
                            else:
                                nc.sync.dma_start(
                                    attnT[kt * 128:(kt + 1) * 128,
                                          2 * hp:2 * hp + 2,
                                          qb * 512:(qb + 1) * 512],
                                    st[:].rearrange("p (t q) -> p t q", q=512))


            if dbg:
                for i in range(2):
                    nc.sync.dma_start(dqh[i], qhT[i][:])
                    nc.sync.dma_start(dkh[i], khT[i][:])
                    nc.sync.dma_start(dctx[i], ctx[i][:])
                for kk in range(NKT):
                    nc.sync.dma_start(dvh[kk], vha[kk][:])

    nc.compile()
    return nc


def _get_nc(causal: bool):
    if causal not in _cache:
        _cache[causal] = _build(causal)
    return _cache[causal]


def _prep_inputs(q, k, v, attn_mask, Wq, bq, Wk, bk, Wv, bv, Wo, bo):
    m2 = np.asarray(attn_mask).reshape(S, S)
    causal = bool(np.array_equal((m2 != 0), np.tril(np.ones((S, S), bool))))

    if causal:
        maskw = np.zeros((NKT, 128, 512), np.float32)
        for kt in range(NKT):
            r = kt % 4
            maskw[kt, :, 0:128 * r] = NEG
            sub = m2[kt * 128:(kt + 1) * 128, kt * 128:(kt + 1) * 128]  # [q,k]
            maskw[kt, :, 128 * r:128 * (r + 1)] = \
                np.where(sub == 0, np.float32(NEG), np.float32(0.0)).T
    else:
        maskw = np.empty((NKT, 128, S), np.float32)
        for kt in range(NKT):
            sub = m2[:, kt * 128:(kt + 1) * 128]  # [q, k]
            maskw[kt] = np.where(sub == 0, np.float32(NEG), np.float32(0.0)).T
    maskw = maskw.astype(np.float16)

    xT = {}
    for name, x in (("xq", q), ("xk", k), ("xv", v)):
        xT[name] = [np.ascontiguousarray(np.asarray(x)[b].T).astype(np.float16)
                    for b in range(B)]

    onesr16 = np.ones((1, 512), np.float16)
    ident16 = np.eye(128, dtype=np.float16)

    Wq, Wk, Wv, Wo = (np.asarray(a, np.float32) for a in (Wq, Wk, Wv, Wo))
    bqv, bkv, bvv, bov = (np.asarray(a, np.float32) for a in (bq, bk, bv, bo))

    in_maps = []
    for c in range(NCORES):
        b = c // 4
        hs = slice(E * (c % 4), E * (c % 4 + 1))
        wvT = Wv[hs, :].T  # [D, 256]
        wv_aug = np.zeros((D, HPC * (DK + 1)), np.float16)
        bv_aug = np.zeros((1, HPC * (DK + 1)), np.float16)
        for h in range(HPC):
            wv_aug[:, (DK + 1) * h:(DK + 1) * h + DK] = \
                wvT[:, DK * h:DK * (h + 1)].astype(np.float16)
            bv_aug[0, (DK + 1) * h:(DK + 1) * h + DK] = \
                bvv[hs][DK * h:DK * (h + 1)].astype(np.float16)
            bv_aug[0, (DK + 1) * h + DK] = 1.0
        im = {
            "xq": xT["xq"][b], "xk": xT["xk"][b], "xv": xT["xv"][b],
            "wq": np.ascontiguousarray(Wq[hs, :].T).astype(np.float16),
            "wk": np.ascontiguousarray(Wk[hs, :].T).astype(np.float16),
            "wv": wv_aug,
            "wo": np.ascontiguousarray(Wo[:, hs].T).astype(np.float16),
            "bq": np.ascontiguousarray(bqv[hs].reshape(2, 128).T),
            "bk": np.ascontiguousarray(bkv[hs].reshape(2, 128).T),
            "bv": bv_aug,
            "bo": (np.ascontiguousarray(bov.reshape(8, 128).T)
                   if c % 4 == 0 else np.zeros((128, 8), np.float32)),
            "onesr": onesr16, "ident": ident16,
            "maskw": maskw,
        }
        in_maps.append(im)
    return causal, in_maps


def _gather(results):
    attn = np.empty((B, H, S, S), np.float32)
    output = np.empty((B, S, D), np.float32)
    for b in range(B):
        acc = None
        for g in range(4):
            c = b * 4 + g
            r = results[c]
            at = r["attnT"]
            for i in range(HPC):
                attn[b, HPC * (c % 4) + i] = at[:, i, :].T.astype(np.float32)
            acc = r["outT"] if acc is None else acc + r["outT"]
        output[b] = acc.T
    return output, attn


def run(trace=False, **inputs):
    causal, in_maps = _prep_inputs(**inputs)
    nc = _get_nc(causal)
    res = bass_utils.run_bass_kernel_spmd(
        nc, in_maps, core_ids=list(range(NCORES)), trace=trace)
    output, attn = _gather(res.results)
    return (output, attn), res


def kernel(**inputs):
    (output, attn), _ = run(trace=False, **inputs)
    return output, attn
